# revision 16
# baseline (speedup 1.0000x reference)
"""AntisymmetricRNN Trainium2 kernel — 8-core data-parallel over batch.

Math (per reference):
    mask = strictly-lower-tri; w_r = v_r * mask; A = w_r - w_r.T
    step:  h' = h + (1/TAU) * tanh( tanh(h) @ A + b_r - GAMMA*h )
           x_pred = tanh(h') @ w_o.T + b_o;   err_t = x_pred - x_t

Design (v5):
  * batch 256 sharded 8 ways (32 per core); recurrence local per core.
  * state layout "h-major": [128 partitions = h%128, free = (h//128, b)] so
    the recurrent matmul output (z^T accumulated per h-out tile into PSUM)
    lands in exactly the state layout -> zero transposes anywhere.
  * per step: 64 bf16 matmuls, lhsT = A 128x128 tiles (stationary, FWL),
    rhs = tanh(h) [128, 32] slices (moving).  The pair issue rate is a hard
    ~26.7ns (LDWEIGHTS xbus floor, dtype-independent - fp8 measured no
    faster), so the schedule exists to keep that stream stall-free.
  * the bias b_r - GAMMA*h enters s NOT via a DVE prewrite of the PSUM bank
    (that put a ~1.3us DVE chain on the z-bank critical path, the v3 main
    stall) but via a tiny identity matmul: v = b_r - GAMMA*h is computed by
    the DVE into an fp16 tile whenever h updates, and a PE matmul
    z[q] += I^T @ v[q] is the LAST accumulator of each chunk.  The A-matmuls
    of the next step then restart the bank with start=True, waiting on
    nothing but the tanh slots.
  * tanh(h) lives in ONE tile TH_all [128, (m=8, slot=8, b=32)] bf16.  An
    8-deep slot ring (two 4-slot phases) lets the output projection of
    steps 4g..4g+3 read a CONTIGUOUS 128-wide stationary slice per k-tile
    while steps 4g+4.. write the other phase - no WAR.  Projection = 8
    matmuls of N=256 (Wo moving) per 4-step group, interleaved into the
    next step's MM stream.
  * recurrence MMs are emitted as 16 (m-pair, k-pair) blocks ordered so the
    4 m-chunks' z finish staggered and consumers of a chunk's th run late
    enough that the previous step's ACT->DVE->ACT chain (~1.0us) has
    landed: thresholds (pairs) K1>=6, K2>=22, K3>=30 vs producer ends
    C = (36, 40, 56, 64).
  * per chunk q: ACT u=tanh(z) (PSUM->PSUM), DVE h+=u/TAU, ACT th=tanh(h)
    (strided dest into TH_all); v-STTs run pairwise off the critical path.
  * fully unrolled (no hardware loops).
"""

import numpy as np
import ml_dtypes
from contextlib import ExitStack

import concourse.bass as bass
import concourse.tile as tile
from concourse import mybir
from concourse.bass_utils import run_bass_kernel_spmd

# ---------------- problem constants (hardcoded per spec) ----------------
S, B, D, H = 512, 256, 256, 1024
NCORES = 8
BS = B // NCORES                  # 32 batch per core
TAU, GAMMA = 10.0, 0.1
INV_TAU = 1.0 / TAU
KT = H // 128                     # 8 contraction tiles
MT = H // 128                     # 8 output tiles
G = 4                             # elementwise chunks per step (2 m-tiles)
CW = (MT // G) * BS               # chunk width in free elems (64)
NSLOT = 8                         # tanh(h) slot ring depth (two 4-phases)
PH = 4                            # slots per projection phase

TRACE = False                     # set True from test harness for profiling
LAST_RESULTS = None               # BassKernelResults stash for the harness

_BUILT = None

# 16 (m-pair P, k-pair K) blocks per step, k-class-major.  Four per-chunk
# tanh chains fire as each chunk's z completes ((0,3)/(1,3)/(2,3)/(3,3)
# staggered), so th chunk q arrives early enough for its consumers next
# step (K0 at the head tolerates chunk0's chain landing ~step start).
BLOCKS = [(0, 0), (1, 0), (2, 0), (3, 0), (0, 1), (1, 1), (2, 1), (3, 1),
          (0, 2), (1, 2), (0, 3), (1, 3), (2, 2), (3, 2), (2, 3), (3, 3)]
# after block index -> projection k-tiles emitted there (on steps j%4==0)
PROJ_AFTER = {1: (0, 1), 5: (2, 3), 12: (4, 5), 14: (6, 7)}


def _split_multi_waits(nc, max_waits: int = 1):
    """The walrus build here supports one sync-wait slot on CTRL-encoded
    instructions; split any multi-wait instruction's extra waits into a chain
    of preceding single-wait NOPs on the same engine (identical semantics)."""
    for fn in nc.m.functions:
        for bb in fn.blocks:
            new_insts = []
            for inst in bb.instructions:
                si = inst.sync_info
                if si is not None and len(si.on_wait) > max_waits:
                    waits = list(si.on_wait)
                    for w in waits[:-max_waits]:
                        nop = mybir.InstNoOp(
                            name=nc.get_next_instruction_name(), ins=[], outs=[])
                        nop.engine = inst.engine
                        nop.sync_info = mybir.SyncInfo(on_wait=[w], on_update=[])
                        nc.register_instruction(nop)
                        new_insts.append(nop)
                    si.on_wait = waits[-max_waits:]
                new_insts.append(inst)
            bb.instructions = new_insts


def _build_bass():
    nc = bass.Bass("TRN2", target_bir_lowering=False, debug=False,
                   num_devices=NCORES)
    dt = mybir.dt
    f32, bf16, f16 = dt.float32, dt.bfloat16, dt.float16

    A_d = nc.dram_tensor("A", [128, KT * MT * 128], bf16, kind="ExternalInput").ap()
    Wo_d = nc.dram_tensor("Wo", [128, KT * D], bf16, kind="ExternalInput").ap()
    Br_d = nc.dram_tensor("Br", [128, MT * BS], f32, kind="ExternalInput").ap()
    h0_d = nc.dram_tensor("h0", [128, MT * BS], f32, kind="ExternalInput").ap()
    th0_d = nc.dram_tensor("th0", [128, MT * BS], bf16, kind="ExternalInput").ap()
    v0_d = nc.dram_tensor("v0", [128, MT * BS], f16, kind="ExternalInput").ap()
    I_d = nc.dram_tensor("I", [128, 128], f16, kind="ExternalInput").ap()
    x_d = nc.dram_tensor("x", [S, BS, D], f32, kind="ExternalInput").ap()
    err_d = nc.dram_tensor("err", [S, BS, D], f32, kind="ExternalOutput").ap()

    Tanh = mybir.ActivationFunctionType.Tanh
    MUL, ADD, SUB = (mybir.AluOpType.mult, mybir.AluOpType.add,
                     mybir.AluOpType.subtract)

    # [S,BS,D] viewed as [S/4, (4*BS)=128, D]: one contiguous 128KB block per
    # 4-step group, partition = (step_low, b).
    x_g = x_d.rearrange("(g s) b d -> g (s b) d", s=PH)
    e_g = err_d.rearrange("(g s) b d -> g (s b) d", s=PH)

    with tile.TileContext(nc) as tc, ExitStack() as ctx:
        const = ctx.enter_context(tc.tile_pool(name="const", bufs=1))
        state = ctx.enter_context(tc.tile_pool(name="state", bufs=1))
        zpool = ctx.enter_context(tc.tile_pool(name="zps", bufs=1, space="PSUM"))
        upool = ctx.enter_context(tc.tile_pool(name="ups", bufs=1, space="PSUM"))
        xppool = ctx.enter_context(tc.tile_pool(name="xpps", bufs=2, space="PSUM"))
        xtp = ctx.enter_context(tc.tile_pool(name="xt", bufs=3))
        etp = ctx.enter_context(tc.tile_pool(name="et", bufs=3))

        A_sb = const.tile([128, KT * MT * 128], bf16, tag="A", name="A_sb")
        Wo_sb = const.tile([128, KT * D], bf16, tag="Wo", name="Wo_sb")
        Br_sb = const.tile([128, MT * BS], f32, tag="Br", name="Br_sb")
        Ident = const.tile([128, 128], f16, tag="I", name="Ident")
        nc.sync.dma_start(A_sb[:], A_d[:])
        nc.sync.dma_start(Wo_sb[:], Wo_d[:])
        nc.sync.dma_start(Br_sb[:], Br_d[:])
        nc.sync.dma_start(Ident[:], I_d[:])

        h_all = state.tile([128, G * CW], f32, tag="h", name="h_all")
        v_all = state.tile([128, G * CW], f16, tag="v", name="v_all")
        # tanh(h): one tile, free = (m, slot, b); step j writes slot j%8.
        TH = state.tile([128, MT * NSLOT * BS], bf16, tag="TH", name="TH")
        TH_v = TH[:].rearrange("p (m s b) -> p m s b", m=MT, s=NSLOT, b=BS)
        z_bufs = [[zpool.tile([128, 2 * CW], f32, tag=f"z{p}{h}",
                               name=f"z_{p}{h}")
                   for h in range(2)] for p in range(2)]
        u_all = upool.tile([128, G * CW], f32, tag="u", name="u_all")
        uT = [u_all[:, q * CW:(q + 1) * CW] for q in range(G)]
        hT = [h_all[:, q * CW:(q + 1) * CW] for q in range(G)]
        vT = [v_all[:, q * CW:(q + 1) * CW] for q in range(G)]

        nc.sync.dma_start(h_all[:], h0_d[:])
        nc.sync.dma_start(v_all[:], v0_d[:])
        for m in range(MT):
            nc.sync.dma_start(TH_v[:, m, NSLOT - 1, :],
                              th0_d[:, m * BS:(m + 1) * BS])

        def th_rhs(k, rd):
            # th k-tile, slot rd: [128, 32]
            off = k * NSLOT * BS + rd * BS
            return TH[:, off:off + BS]

        def emit_vmm(j, q):
            # bias add: z chunk q += I^T @ v[q]  (an accumulator of the bank)
            zb = z_bufs[j % 2][q // 2]
            lo = (q % 2) * CW
            nc.tensor.matmul(zb[:, lo:lo + CW], lhsT=Ident[:], rhs=vT[q],
                             start=False, stop=True, skip_group_check=True)

        def emit_u(j, q, wide):
            # u = tanh(s) for chunk q (wide=True: chunks q..q+1, one 128 ACT)
            zb = z_bufs[j % 2][q // 2]
            lo = (q % 2) * CW
            w = 2 * CW if wide else CW
            nc.scalar.activation(u_all[:, q * CW:q * CW + w],
                                 zb[:, lo:lo + w], Tanh)

        def emit_hth(j, q):
            # h += u/TAU [DVE]; th = tanh(h) [strided ACT]
            wr = j % NSLOT
            nc.vector.scalar_tensor_tensor(
                hT[q], uT[q], INV_TAU, hT[q], MUL, ADD)
            nc.scalar.activation(
                TH_v[:, 2 * q:2 * q + 2, wr, :],
                hT[q].rearrange("p (m b) -> p m b", m=2), Tanh)

        def emit_vstt(j, half):
            # v = b_r - GAMMA*h for half (feeds next step's bias v-MMs)
            if j >= S - 1:
                return
            lo, hi = 2 * half * CW, (2 * half + 2) * CW
            nc.vector.scalar_tensor_tensor(
                v_all[:, lo:hi], h_all[:, lo:hi], -GAMMA,
                Br_sb[:, lo:hi], MUL, ADD)

        def emit_proj_mm(g, k, xp):
            # xp[(s,b), d] += th[k-tile](slots of phase, b)^T @ Wo[k]
            ph = (g % 2) * PH * BS
            nc.tensor.matmul(
                xp[:],
                lhsT=TH[:, k * NSLOT * BS + ph:k * NSLOT * BS + ph + PH * BS],
                rhs=Wo_sb[:, k * D:(k + 1) * D],
                start=(k == 0), stop=(k == KT - 1), skip_group_check=True)

        def emit_proj_tail(g, xp):
            xt = xtp.tile([128, D], f32, tag="xt", name="xt")
            nc.sync.dma_start(xt[:], x_g[g])
            et = etp.tile([128, D], f32, tag="et", name="et")
            nc.vector.scalar_tensor_tensor(                    # xp - (x - b_o)
                et[:], xp[:], 0.0, xt[:], ADD, SUB)
            nc.sync.dma_start(e_g[g], et[:])

        for j in range(S):
            rd = (j - 1) % NSLOT
            do_proj = (j % PH == 0 and j > 0)
            if do_proj:
                g = j // PH - 1
                xp = xppool.tile([128, D], f32, tag="xp", name="xp")
            for bi, (P, K) in enumerate(BLOCKS):
                for mo in range(2):
                    m = 2 * P + mo
                    for ko in range(2):
                        k = 2 * K + ko
                        nc.tensor.matmul(
                            z_bufs[j % 2][P // 2][:, (P % 2) * CW + mo * BS:
                                                  (P % 2) * CW + (mo + 1) * BS],
                            lhsT=A_sb[:, (k * MT + m) * 128:(k * MT + m + 1) * 128],
                            rhs=th_rhs(k, rd),
                            start=(k == 0), stop=False,
                            skip_group_check=True)
                if do_proj and bi in PROJ_AFTER:
                    for k in PROJ_AFTER[bi]:
                        emit_proj_mm(g, k, xp)
                    if PROJ_AFTER[bi][1] == KT - 1:
                        emit_proj_tail(g, xp)
                if bi == 10:
                    emit_vmm(j, 0)
                elif bi == 11:
                    emit_vmm(j, 1)
                    emit_u(j, 0, wide=True)     # u chunks 0+1
                    emit_hth(j, 0)
                elif bi == 12:
                    emit_hth(j, 1)
                    emit_vstt(j, 0)
                elif bi == 14:
                    emit_vmm(j, 2)
                    emit_u(j, 2, wide=False)
                    emit_hth(j, 2)
                elif bi == 15:
                    emit_vmm(j, 3)
                    emit_u(j, 3, wide=False)
                    emit_hth(j, 3)
                    emit_vstt(j, 1)

        # final group's projection (steps 508..511)
        g = S // PH - 1
        xp = xppool.tile([128, D], f32, tag="xp", name="xp")
        for k in range(KT):
            emit_proj_mm(g, k, xp)
        emit_proj_tail(g, xp)

    _split_multi_waits(nc)
    return nc


def _host_prep(x, h_init, v_r, b_r, w_o, b_o):
    """Build per-core input maps (all layout work in numpy)."""
    x = np.asarray(x, np.float32)
    h_init = np.asarray(h_init, np.float32)
    v_r = np.asarray(v_r, np.float32)
    b_r = np.asarray(b_r, np.float32)
    w_o = np.asarray(w_o, np.float32)
    b_o = np.asarray(b_o, np.float32)

    mask = np.tril(np.ones((H, H), np.float32), -1)
    w_r = v_r * mask
    A = w_r - w_r.T                                           # [H, H]
    # A_sb[p, (k*MT+m)*128 + c] = A[k*128+p, m*128+c]
    A_sb = np.ascontiguousarray(
        A.reshape(KT, 128, MT, 128).transpose(1, 0, 2, 3).reshape(128, KT * MT * 128)
    ).astype(ml_dtypes.bfloat16)
    # Wo_sb[p, k*D + d] = w_o[d, k*128+p]   (w_o^T tiles, moving operand)
    Wo_sb = np.ascontiguousarray(
        w_o.T.reshape(KT, 128, D).transpose(1, 0, 2).reshape(128, KT * D)
    ).astype(ml_dtypes.bfloat16)
    # Br[p, m*BS+b] = b_r[m*128+p]
    Br = np.ascontiguousarray(
        np.broadcast_to(b_r.reshape(MT, 128, 1).transpose(1, 0, 2), (128, MT, BS))
    ).reshape(128, MT * BS).astype(np.float32)

    in_maps = []
    for c in range(NCORES):
        hc = h_init[c * BS:(c + 1) * BS]                       # [BS, H]
        h0 = np.ascontiguousarray(
            hc.reshape(BS, MT, 128).transpose(2, 1, 0)         # [128, MT, BS]
        ).reshape(128, MT * BS).astype(np.float32)
        th0 = np.tanh(h0)
        v0 = Br - GAMMA * h0
        in_maps.append({
            "A": A_sb, "Wo": Wo_sb, "Br": Br,
            "h0": h0, "th0": th0.astype(ml_dtypes.bfloat16),
            "v0": v0.astype(np.float16),
            "I": np.eye(128, dtype=np.float16),
            "x": np.ascontiguousarray(x[:, c * BS:(c + 1) * BS, :] - b_o),
        })
    return in_maps


def kernel(x, h_init, v_r, b_r, w_o, b_o):
    global _BUILT, LAST_RESULTS
    if _BUILT is None:
        _BUILT = _build_bass()
    nc = _BUILT
    in_maps = _host_prep(x, h_init, v_r, b_r, w_o, b_o)
    res = run_bass_kernel_spmd(nc, in_maps, core_ids=list(range(NCORES)),
                               trace=TRACE)
    LAST_RESULTS = res
    out = np.empty((S, B, D), np.float32)
    for c in range(NCORES):
        out[:, c * BS:(c + 1) * BS, :] = np.asarray(res.results[c]["err"])
    return out


# revision 17
# speedup vs baseline: 1.1899x; 1.1899x over previous
"""AntisymmetricRNN Trainium2 kernel — 8-core data-parallel over batch.

Math (per reference):
    mask = strictly-lower-tri; w_r = v_r * mask; A = w_r - w_r.T
    step:  h' = h + (1/TAU) * tanh( tanh(h) @ A + b_r - GAMMA*h )
           x_pred = tanh(h') @ w_o.T + b_o;   err_t = x_pred - x_t

Design (v5):
  * batch 256 sharded 8 ways (32 per core); recurrence local per core.
  * state layout "h-major": [128 partitions = h%128, free = (h//128, b)] so
    the recurrent matmul output (z^T accumulated per h-out tile into PSUM)
    lands in exactly the state layout -> zero transposes anywhere.
  * per step: 64 bf16 matmuls, lhsT = A 128x128 tiles (stationary, FWL),
    rhs = tanh(h) [128, 32] slices (moving).  The pair issue rate is a hard
    ~26.7ns (LDWEIGHTS xbus floor, dtype-independent - fp8 measured no
    faster), so the schedule exists to keep that stream stall-free.
  * the bias b_r - GAMMA*h enters s NOT via a DVE prewrite of the PSUM bank
    (that put a ~1.3us DVE chain on the z-bank critical path, the v3 main
    stall) but via a tiny identity matmul: v = b_r - GAMMA*h is computed by
    the DVE into an fp16 tile whenever h updates, and a PE matmul
    z[q] += I^T @ v[q] is the LAST accumulator of each chunk.  The A-matmuls
    of the next step then restart the bank with start=True, waiting on
    nothing but the tanh slots.
  * tanh(h) lives in ONE tile TH_all [128, (m=8, slot=8, b=32)] bf16.  An
    8-deep slot ring (two 4-slot phases) lets the output projection of
    steps 4g..4g+3 read a CONTIGUOUS 128-wide stationary slice per k-tile
    while steps 4g+4.. write the other phase - no WAR.  Projection = 8
    matmuls of N=256 (Wo moving) per 4-step group, interleaved into the
    next step's MM stream.
  * recurrence MMs are emitted as 16 (m-pair, k-pair) blocks ordered so the
    4 m-chunks' z finish staggered and consumers of a chunk's th run late
    enough that the previous step's ACT->DVE->ACT chain (~1.0us) has
    landed: thresholds (pairs) K1>=6, K2>=22, K3>=30 vs producer ends
    C = (36, 40, 56, 64).
  * per chunk q: ACT u=tanh(z) (PSUM->PSUM), DVE h+=u/TAU, ACT th=tanh(h)
    (strided dest into TH_all); v-STTs run pairwise off the critical path.
  * fully unrolled (no hardware loops).
"""

import numpy as np
import ml_dtypes
from contextlib import ExitStack

import concourse.bass as bass
import concourse.tile as tile
from concourse import mybir
from concourse.bass_utils import run_bass_kernel_spmd

# ---------------- problem constants (hardcoded per spec) ----------------
S, B, D, H = 512, 256, 256, 1024
NCORES = 8
BS = B // NCORES                  # 32 batch per core
TAU, GAMMA = 10.0, 0.1
INV_TAU = 1.0 / TAU
KT = H // 128                     # 8 contraction tiles
MT = H // 128                     # 8 output tiles
G = 4                             # elementwise chunks per step (2 m-tiles)
CW = (MT // G) * BS               # chunk width in free elems (64)
NSLOT = 8                         # tanh(h) slot ring depth (two 4-phases)
PH = 4                            # slots per projection phase

TRACE = False                     # set True from test harness for profiling
LAST_RESULTS = None               # BassKernelResults stash for the harness

_BUILT = None

# 16 (m-pair P, k-pair K) blocks per step (v7 interleave): half A (P0/P1)
# completes by block 10, half B at the end; each half's u+tanh chain is
# emitted at its completion point, with the bias v-MM as the half's final
# accumulator.
BLOCKS = [(0, 0), (1, 0), (2, 0), (3, 0), (0, 1), (1, 1), (0, 2), (1, 2),
          (0, 3), (1, 3), (2, 1), (3, 1), (2, 2), (3, 2), (2, 3), (3, 3)]
# after block index -> projection k-tiles emitted there (on steps j%4==0)
PROJ_AFTER = {1: (0, 1), 5: (2, 3), 11: (4, 5), 13: (6, 7)}


def _split_multi_waits(nc, max_waits: int = 1):
    """The walrus build here supports one sync-wait slot on CTRL-encoded
    instructions; split any multi-wait instruction's extra waits into a chain
    of preceding single-wait NOPs on the same engine (identical semantics)."""
    for fn in nc.m.functions:
        for bb in fn.blocks:
            new_insts = []
            for inst in bb.instructions:
                si = inst.sync_info
                if si is not None and len(si.on_wait) > max_waits:
                    waits = list(si.on_wait)
                    for w in waits[:-max_waits]:
                        nop = mybir.InstNoOp(
                            name=nc.get_next_instruction_name(), ins=[], outs=[])
                        nop.engine = inst.engine
                        nop.sync_info = mybir.SyncInfo(on_wait=[w], on_update=[])
                        nc.register_instruction(nop)
                        new_insts.append(nop)
                    si.on_wait = waits[-max_waits:]
                new_insts.append(inst)
            bb.instructions = new_insts


def _build_bass():
    nc = bass.Bass("TRN2", target_bir_lowering=False, debug=False,
                   num_devices=NCORES)
    dt = mybir.dt
    f32, bf16, f16 = dt.float32, dt.bfloat16, dt.float16

    A_d = nc.dram_tensor("A", [128, KT * MT * 128], bf16, kind="ExternalInput").ap()
    Wo_d = nc.dram_tensor("Wo", [128, KT * D], bf16, kind="ExternalInput").ap()
    Br_d = nc.dram_tensor("Br", [128, MT * BS], f32, kind="ExternalInput").ap()
    h0_d = nc.dram_tensor("h0", [128, MT * BS], f32, kind="ExternalInput").ap()
    th0_d = nc.dram_tensor("th0", [128, MT * BS], bf16, kind="ExternalInput").ap()
    v0_d = nc.dram_tensor("v0", [128, MT * BS], f16, kind="ExternalInput").ap()
    I_d = nc.dram_tensor("I", [128, 128], f16, kind="ExternalInput").ap()
    x_d = nc.dram_tensor("x", [S, BS, D], f32, kind="ExternalInput").ap()
    err_d = nc.dram_tensor("err", [S, BS, D], f32, kind="ExternalOutput").ap()

    Tanh = mybir.ActivationFunctionType.Tanh
    MUL, ADD, SUB = (mybir.AluOpType.mult, mybir.AluOpType.add,
                     mybir.AluOpType.subtract)

    # [S,BS,D] viewed as [S/4, (4*BS)=128, D]: one contiguous 128KB block per
    # 4-step group, partition = (step_low, b).
    x_g = x_d.rearrange("(g s) b d -> g (s b) d", s=PH)
    e_g = err_d.rearrange("(g s) b d -> g (s b) d", s=PH)

    with tile.TileContext(nc) as tc, ExitStack() as ctx:
        const = ctx.enter_context(tc.tile_pool(name="const", bufs=1))
        state = ctx.enter_context(tc.tile_pool(name="state", bufs=1))
        zpool = ctx.enter_context(tc.tile_pool(name="zps", bufs=1, space="PSUM"))
        upool = ctx.enter_context(tc.tile_pool(name="ups", bufs=1, space="PSUM"))
        xppool = ctx.enter_context(tc.tile_pool(name="xpps", bufs=2, space="PSUM"))
        xtp = ctx.enter_context(tc.tile_pool(name="xt", bufs=3))
        etp = ctx.enter_context(tc.tile_pool(name="et", bufs=3))

        A_sb = const.tile([128, KT * MT * 128], bf16, tag="A", name="A_sb")
        Wo_sb = const.tile([128, KT * D], bf16, tag="Wo", name="Wo_sb")
        Br_sb = const.tile([128, MT * BS], f32, tag="Br", name="Br_sb")
        Ident = const.tile([128, 128], f16, tag="I", name="Ident")
        nc.sync.dma_start(A_sb[:], A_d[:])
        nc.sync.dma_start(Wo_sb[:], Wo_d[:])
        nc.sync.dma_start(Br_sb[:], Br_d[:])
        nc.sync.dma_start(Ident[:], I_d[:])

        h_all = state.tile([128, G * CW], f32, tag="h", name="h_all")
        v_all = state.tile([128, G * CW], f16, tag="v", name="v_all")
        # tanh(h): one tile, free = (m, slot, b); step j writes slot j%8.
        TH = state.tile([128, MT * NSLOT * BS], bf16, tag="TH", name="TH")
        TH_v = TH[:].rearrange("p (m s b) -> p m s b", m=MT, s=NSLOT, b=BS)
        z_bufs = [[zpool.tile([128, 2 * CW], f32, tag=f"z{p}{h}",
                               name=f"z_{p}{h}")
                   for h in range(2)] for p in range(2)]
        u_all = upool.tile([128, G * CW], f32, tag="u", name="u_all")
        uT = [u_all[:, q * CW:(q + 1) * CW] for q in range(G)]
        hT = [h_all[:, q * CW:(q + 1) * CW] for q in range(G)]
        vT = [v_all[:, q * CW:(q + 1) * CW] for q in range(G)]

        nc.sync.dma_start(h_all[:], h0_d[:])
        nc.sync.dma_start(v_all[:], v0_d[:])
        for m in range(MT):
            nc.sync.dma_start(TH_v[:, m, NSLOT - 1, :],
                              th0_d[:, m * BS:(m + 1) * BS])

        def th_rhs(k, rd):
            # th k-tile, slot rd: [128, 32]
            off = k * NSLOT * BS + rd * BS
            return TH[:, off:off + BS]

        def emit_vmm(j, q):
            # bias add: z chunk q += I^T @ v[q]  (an accumulator of the bank)
            zb = z_bufs[j % 2][q // 2]
            lo = (q % 2) * CW
            nc.tensor.matmul(zb[:, lo:lo + CW], lhsT=Ident[:], rhs=vT[q],
                             start=False, stop=True, skip_group_check=True)

        def emit_u(j, q, wide):
            # u = tanh(s) for chunk q (wide=True: chunks q..q+1, one 128 ACT)
            zb = z_bufs[j % 2][q // 2]
            lo = (q % 2) * CW
            w = 2 * CW if wide else CW
            nc.scalar.activation(u_all[:, q * CW:q * CW + w],
                                 zb[:, lo:lo + w], Tanh)

        def emit_hth(j, q):
            # h += u/TAU [DVE]; th = tanh(h) [strided ACT]
            wr = j % NSLOT
            nc.vector.scalar_tensor_tensor(
                hT[q], uT[q], INV_TAU, hT[q], MUL, ADD)
            nc.scalar.activation(
                TH_v[:, 2 * q:2 * q + 2, wr, :],
                hT[q].rearrange("p (m b) -> p m b", m=2), Tanh)

        def emit_vstt(j, half):
            # v = b_r - GAMMA*h for half (feeds next step's bias v-MMs)
            if j >= S - 1:
                return
            lo, hi = 2 * half * CW, (2 * half + 2) * CW
            nc.vector.scalar_tensor_tensor(
                v_all[:, lo:hi], h_all[:, lo:hi], -GAMMA,
                Br_sb[:, lo:hi], MUL, ADD)

        def emit_proj_mm(g, k, xp):
            # xp[(s,b), d] += th[k-tile](slots of phase, b)^T @ Wo[k]
            ph = (g % 2) * PH * BS
            nc.tensor.matmul(
                xp[:],
                lhsT=TH[:, k * NSLOT * BS + ph:k * NSLOT * BS + ph + PH * BS],
                rhs=Wo_sb[:, k * D:(k + 1) * D],
                start=(k == 0), stop=(k == KT - 1), skip_group_check=True)

        def emit_proj_tail(g, xp):
            xt = xtp.tile([128, D], f32, tag="xt", name="xt")
            nc.sync.dma_start(xt[:], x_g[g])
            et = etp.tile([128, D], f32, tag="et", name="et")
            nc.vector.scalar_tensor_tensor(                    # xp - (x - b_o)
                et[:], xp[:], 0.0, xt[:], ADD, SUB)
            nc.sync.dma_start(e_g[g], et[:])

        for j in range(S):
            rd = (j - 1) % NSLOT
            do_proj = (j % PH == 0 and j > 0)
            if do_proj:
                g = j // PH - 1
                xp = xppool.tile([128, D], f32, tag="xp", name="xp")
            for bi, (P, K) in enumerate(BLOCKS):
                for mo in range(2):
                    m = 2 * P + mo
                    for ko in range(2):
                        k = 2 * K + ko
                        nc.tensor.matmul(
                            z_bufs[j % 2][P // 2][:, (P % 2) * CW + mo * BS:
                                                  (P % 2) * CW + (mo + 1) * BS],
                            lhsT=A_sb[:, (k * MT + m) * 128:(k * MT + m + 1) * 128],
                            rhs=th_rhs(k, rd),
                            start=(k == 0), stop=False,
                            skip_group_check=True)
                if do_proj and bi in PROJ_AFTER:
                    for k in PROJ_AFTER[bi]:
                        emit_proj_mm(g, k, xp)
                    if PROJ_AFTER[bi][1] == KT - 1:
                        emit_proj_tail(g, xp)
                if bi == 9:
                    emit_vmm(j, 0)
                    emit_vmm(j, 1)
                    emit_u(j, 0, wide=True)     # u chunks 0+1
                    emit_hth(j, 0)
                    emit_hth(j, 1)
                    emit_vstt(j, 0)
                elif bi == 15:
                    emit_vmm(j, 2)
                    emit_vmm(j, 3)
                    emit_u(j, 2, wide=True)     # u chunks 2+3
                    emit_hth(j, 2)
                    emit_hth(j, 3)
                    emit_vstt(j, 1)


        # final group's projection (steps 508..511)
        g = S // PH - 1
        xp = xppool.tile([128, D], f32, tag="xp", name="xp")
        for k in range(KT):
            emit_proj_mm(g, k, xp)
        emit_proj_tail(g, xp)

    _split_multi_waits(nc)
    return nc


def _host_prep(x, h_init, v_r, b_r, w_o, b_o):
    """Build per-core input maps (all layout work in numpy)."""
    x = np.asarray(x, np.float32)
    h_init = np.asarray(h_init, np.float32)
    v_r = np.asarray(v_r, np.float32)
    b_r = np.asarray(b_r, np.float32)
    w_o = np.asarray(w_o, np.float32)
    b_o = np.asarray(b_o, np.float32)

    mask = np.tril(np.ones((H, H), np.float32), -1)
    w_r = v_r * mask
    A = w_r - w_r.T                                           # [H, H]
    # A_sb[p, (k*MT+m)*128 + c] = A[k*128+p, m*128+c]
    A_sb = np.ascontiguousarray(
        A.reshape(KT, 128, MT, 128).transpose(1, 0, 2, 3).reshape(128, KT * MT * 128)
    ).astype(ml_dtypes.bfloat16)
    # Wo_sb[p, k*D + d] = w_o[d, k*128+p]   (w_o^T tiles, moving operand)
    Wo_sb = np.ascontiguousarray(
        w_o.T.reshape(KT, 128, D).transpose(1, 0, 2).reshape(128, KT * D)
    ).astype(ml_dtypes.bfloat16)
    # Br[p, m*BS+b] = b_r[m*128+p]
    Br = np.ascontiguousarray(
        np.broadcast_to(b_r.reshape(MT, 128, 1).transpose(1, 0, 2), (128, MT, BS))
    ).reshape(128, MT * BS).astype(np.float32)

    in_maps = []
    for c in range(NCORES):
        hc = h_init[c * BS:(c + 1) * BS]                       # [BS, H]
        h0 = np.ascontiguousarray(
            hc.reshape(BS, MT, 128).transpose(2, 1, 0)         # [128, MT, BS]
        ).reshape(128, MT * BS).astype(np.float32)
        th0 = np.tanh(h0)
        v0 = Br - GAMMA * h0
        in_maps.append({
            "A": A_sb, "Wo": Wo_sb, "Br": Br,
            "h0": h0, "th0": th0.astype(ml_dtypes.bfloat16),
            "v0": v0.astype(np.float16),
            "I": np.eye(128, dtype=np.float16),
            "x": np.ascontiguousarray(x[:, c * BS:(c + 1) * BS, :] - b_o),
        })
    return in_maps


def kernel(x, h_init, v_r, b_r, w_o, b_o):
    global _BUILT, LAST_RESULTS
    if _BUILT is None:
        _BUILT = _build_bass()
    nc = _BUILT
    in_maps = _host_prep(x, h_init, v_r, b_r, w_o, b_o)
    res = run_bass_kernel_spmd(nc, in_maps, core_ids=list(range(NCORES)),
                               trace=TRACE)
    LAST_RESULTS = res
    out = np.empty((S, B, D), np.float32)
    for c in range(NCORES):
        out[:, c * BS:(c + 1) * BS, :] = np.asarray(res.results[c]["err"])
    return out
